# revision 5
# baseline (speedup 1.0000x reference)
"""DeepSeek-style MoE layer (64 routed experts, top-6 grouped routing, 2
shared experts) on 8 Trainium2 NeuronCores.

Strategy (expert-parallel, slot-tile formulation):
  * Host computes the gate/grouped-top-k routing exactly in fp32 numpy and
    packs each expert's assigned tokens into "slot tiles" of <=128 tokens.
    An expert with load > 128 contributes multiple tiles.  Tiles are
    bin-packed across the 8 cores (weights of a tile's expert are streamed
    to the core that owns the tile), NS tiles per core, uniform so one SPMD
    module serves all cores.
  * Device per core: for each slot tile, gate/up projections
    (h1T = wgT @ xT form, I on partitions), SiLU * up, down projection
    (Y[slot, H]), then a combine matmul  out[T,H] += ST[slot,T].T @ Y
    where ST carries the (renormalized * routed_scaling_factor) combine
    weights scattered per token.  The shared experts are tensor-parallel
    across cores (SI=1024 split into 8 slices of 128) and accumulate into
    the same PSUM tiles.  Everything is bf16 in / fp32 accumulate.
  * Host sums the 8 partial [T,H] outputs (the expert-parallel unshard).
"""
import numpy as np
import ml_dtypes

import concourse.bacc as bacc
import concourse.mybir as mybir
from concourse import tile
from concourse.bass_utils import run_bass_kernel_spmd

BF16 = ml_dtypes.bfloat16

T, H, E, I = 1024, 1024, 64, 512
NG, TKG, TOPK = 8, 3, 6
RSF = 2.5
P = 128
KT = H // P          # 8 k-tiles over hidden dim
IT = I // P          # 4 tiles over intermediate dim
HN = H // 512        # 2 output column chunks
TN = T // P          # 8 token tiles
N_CORES = 8
SI_SLICE = P         # shared-expert intermediate slice per core (2*512/8)


# ---------------------------------------------------------------- routing
def _route(x, gate_w, e_bias):
    """Exact fp32 replica of the reference noaux_tc grouped top-k."""
    logits = x.astype(np.float32) @ gate_w.astype(np.float32)
    scores = 1.0 / (1.0 + np.exp(-logits))
    sc = scores + e_bias
    g = sc.reshape(T, NG, E // NG)
    top2 = np.sort(g, axis=-1)[:, :, -2:].sum(-1)
    gidx = np.argsort(-top2, axis=-1)[:, :TKG]
    gmask = np.zeros((T, NG), bool)
    gmask[np.arange(T)[:, None], gidx] = True
    emask = np.repeat(gmask, E // NG, axis=1)
    masked = np.where(emask, sc, -np.inf)
    ids = np.argsort(-masked, axis=-1)[:, :TOPK]
    w = np.take_along_axis(scores, ids, axis=1)
    w = w / w.sum(-1, keepdims=True)
    return ids, w


def _pack(ids, w):
    """Split each expert's token list into tiles of <=P slots and bin-pack
    the tiles across cores (balanced by tile count, then token count)."""
    tiles = []
    for e in range(E):
        rows, cols = np.nonzero(ids == e)
        wts = w[rows, cols] * RSF
        for off in range(0, len(rows), P):
            tiles.append((e, rows[off:off + P], wts[off:off + P]))
    cores = [[] for _ in range(N_CORES)]
    for t in sorted(tiles, key=lambda z: -len(z[1])):
        c = min(range(N_CORES),
                key=lambda i: (len(cores[i]), sum(len(z[1]) for z in cores[i])))
        cores[c].append(t)
    ns = max(len(c) for c in cores)
    return cores, ns


def _prepare(inputs):
    """Host-side shard/dispatch: returns (in_maps, NS)."""
    x = np.asarray(inputs["hidden_states"], np.float32)
    w_gate = np.asarray(inputs["w_gate"], np.float32)
    w_up = np.asarray(inputs["w_up"], np.float32)
    w_down = np.asarray(inputs["w_down"], np.float32)
    sw_gate = np.asarray(inputs["sw_gate"], np.float32)
    sw_up = np.asarray(inputs["sw_up"], np.float32)
    sw_down = np.asarray(inputs["sw_down"], np.float32)

    ids, w = _route(x, np.asarray(inputs["gate_w"], np.float32),
                    np.asarray(inputs["e_bias"], np.float32))
    cores, ns = _pack(ids, w)

    # xt layout [P, KT*T]: xt[p, k*T + t] = x[t, k*P + p]
    xt = np.ascontiguousarray(
        x.T.reshape(KT, P, T).transpose(1, 0, 2).reshape(P, KT * T)).astype(BF16)
    # pre-reshaped expert weights in sbuf layout
    wg_sb_all = np.ascontiguousarray(
        w_gate.reshape(E, KT, P, I).transpose(0, 2, 1, 3).reshape(E, P, KT * I)).astype(BF16)
    wu_sb_all = np.ascontiguousarray(
        w_up.reshape(E, KT, P, I).transpose(0, 2, 1, 3).reshape(E, P, KT * I)).astype(BF16)
    wd_sb_all = np.ascontiguousarray(
        w_down.reshape(E, IT, P, H).transpose(0, 2, 1, 3).reshape(E, P, IT * H)).astype(BF16)

    in_maps = []
    for c in range(N_CORES):
        ctiles = cores[c]
        xg = np.zeros((KT, P, ns * P), np.float32)
        st = np.zeros((ns, P, T), BF16)
        wg_t = np.zeros((ns, P, KT * I), BF16)
        wu_t = np.zeros((ns, P, KT * I), BF16)
        wd_t = np.zeros((ns, P, IT * H), BF16)
        for s, (e, toks, wts) in enumerate(ctiles):
            n = len(toks)
            if n:
                xg[:, :, s * P:s * P + n] = x[toks].T.reshape(KT, P, n)
                st[s, np.arange(n), toks] = wts.astype(BF16)
            wg_t[s] = wg_sb_all[e]
            wu_t[s] = wu_sb_all[e]
            wd_t[s] = wd_sb_all[e]
        # xg layout [P, KT*ns*P]: xg2[p, k*ns*P + col] = x[tok(col), k*P + p]
        xg2 = np.ascontiguousarray(
            xg.transpose(1, 0, 2).reshape(P, KT * ns * P)).astype(BF16)
        # st layout [P, ns*T]
        st2 = np.ascontiguousarray(
            st.transpose(1, 0, 2).reshape(P, ns * T))
        sl = slice(c * SI_SLICE, (c + 1) * SI_SLICE)
        swg = np.ascontiguousarray(
            sw_gate[:, sl].reshape(KT, P, SI_SLICE).transpose(1, 0, 2)
            .reshape(P, KT * SI_SLICE)).astype(BF16)
        swu = np.ascontiguousarray(
            sw_up[:, sl].reshape(KT, P, SI_SLICE).transpose(1, 0, 2)
            .reshape(P, KT * SI_SLICE)).astype(BF16)
        in_maps.append({
            "xg": xg2,
            "st": st2,
            "wg": wg_t,
            "wu": wu_t,
            "wd": wd_t,
            "xt": xt,
            "swg": swg,
            "swu": swu,
            "swd": sw_down[sl, :].astype(BF16),
        })
    return in_maps, ns


# ----------------------------------------------------------------- device
def _build(ns):
    BF = mybir.dt.bfloat16
    F32 = mybir.dt.float32
    SILU = mybir.ActivationFunctionType.Silu

    nc = bacc.Bacc(None, target_bir_lowering=False)
    xg_d = nc.declare_dram_parameter("xg", [P, KT * ns * P], BF, isOutput=False)
    st_d = nc.declare_dram_parameter("st", [P, ns * T], BF, isOutput=False)
    wg_d = nc.declare_dram_parameter("wg", [ns, P, KT * I], BF, isOutput=False)
    wu_d = nc.declare_dram_parameter("wu", [ns, P, KT * I], BF, isOutput=False)
    wd_d = nc.declare_dram_parameter("wd", [ns, P, IT * H], BF, isOutput=False)
    xt_d = nc.declare_dram_parameter("xt", [P, KT * T], BF, isOutput=False)
    swg_d = nc.declare_dram_parameter("swg", [P, KT * SI_SLICE], BF, isOutput=False)
    swu_d = nc.declare_dram_parameter("swu", [P, KT * SI_SLICE], BF, isOutput=False)
    swd_d = nc.declare_dram_parameter("swd", [SI_SLICE, H], BF, isOutput=False)
    out_d = nc.declare_dram_parameter("out", [T, H], F32, isOutput=True)

    with tile.TileContext(nc) as tc:
        with tc.tile_pool(name="big", bufs=1) as big, \
             tc.tile_pool(name="wpool", bufs=2) as wpool, \
             tc.tile_pool(name="hpool", bufs=3) as hpool, \
             tc.tile_pool(name="opool", bufs=3) as opool, \
             tc.tile_pool(name="gup", bufs=4, space="PSUM") as gup, \
             tc.tile_pool(name="ypsum", bufs=2, space="PSUM") as ypsum, \
             tc.tile_pool(name="cpsum", bufs=2, space="PSUM") as cpsum:

            xg_sb = big.tile([P, KT * ns * P], BF, tag="xg")
            nc.sync.dma_start(out=xg_sb[:], in_=xg_d[:])
            xt_sb = big.tile([P, KT * T], BF, tag="xt")
            nc.sync.dma_start(out=xt_sb[:], in_=xt_d[:])
            st_sb = big.tile([P, ns * T], BF, tag="st")
            nc.sync.dma_start(out=st_sb[:], in_=st_d[:])
            swg_sb = big.tile([P, KT * SI_SLICE], BF, tag="swg")
            nc.sync.dma_start(out=swg_sb[:], in_=swg_d[:])
            swu_sb = big.tile([P, KT * SI_SLICE], BF, tag="swu")
            nc.sync.dma_start(out=swu_sb[:], in_=swu_d[:])
            swd_sb = big.tile([P, H], BF, tag="swd")
            nc.sync.dma_start(out=swd_sb[:], in_=swd_d[:])
            y_sb = big.tile([P, ns * H], BF, tag="y")
            hsh_sb = big.tile([P, T], BF, tag="hsh")

            # ---- shared experts (TP slice of SI on this core)
            for tn in range(T // 512):
                pg = gup.tile([P, 512], F32, tag="pg")
                for k in range(KT):
                    nc.tensor.matmul(
                        pg[:],
                        swg_sb[:, k * SI_SLICE:(k + 1) * SI_SLICE],
                        xt_sb[:, k * T + tn * 512: k * T + tn * 512 + 512],
                        start=(k == 0), stop=(k == KT - 1))
                hg = hpool.tile([P, 512], F32, tag="hg")
                nc.scalar.activation(hg[:], pg[:], SILU)
                pu = gup.tile([P, 512], F32, tag="pg")
                for k in range(KT):
                    nc.tensor.matmul(
                        pu[:],
                        swu_sb[:, k * SI_SLICE:(k + 1) * SI_SLICE],
                        xt_sb[:, k * T + tn * 512: k * T + tn * 512 + 512],
                        start=(k == 0), stop=(k == KT - 1))
                nc.vector.tensor_mul(hsh_sb[:, tn * 512:(tn + 1) * 512], hg[:], pu[:])

            # ---- routed experts, one slot tile (<=128 tokens, one expert) at a time
            for s in range(ns):
                wgs = wpool.tile([P, KT * I], BF, tag="wg")
                nc.sync.dma_start(out=wgs[:], in_=wg_d[s])
                wus = wpool.tile([P, KT * I], BF, tag="wu")
                nc.sync.dma_start(out=wus[:], in_=wu_d[s])
                wds = wpool.tile([P, IT * H], BF, tag="wd")
                nc.sync.dma_start(out=wds[:], in_=wd_d[s])

                pg = gup.tile([P, I], F32, tag="pg")
                for isl in range(IT):
                    for k in range(KT):
                        nc.tensor.matmul(
                            pg[:, isl * P:(isl + 1) * P],
                            wgs[:, k * I + isl * P: k * I + (isl + 1) * P],
                            xg_sb[:, (k * ns + s) * P: (k * ns + s + 1) * P],
                            start=(k == 0), stop=(k == KT - 1))
                hg = hpool.tile([P, I], F32, tag="hg")
                nc.scalar.activation(hg[:], pg[:], SILU)
                pu = gup.tile([P, I], F32, tag="pg")
                for isl in range(IT):
                    for k in range(KT):
                        nc.tensor.matmul(
                            pu[:, isl * P:(isl + 1) * P],
                            wus[:, k * I + isl * P: k * I + (isl + 1) * P],
                            xg_sb[:, (k * ns + s) * P: (k * ns + s + 1) * P],
                            start=(k == 0), stop=(k == KT - 1))
                hb = hpool.tile([P, I], BF, tag="hb")
                nc.vector.tensor_mul(hb[:], hg[:], pu[:])
                for hn in range(HN):
                    py = ypsum.tile([P, 512], F32, tag="py")
                    for isl in range(IT):
                        nc.tensor.matmul(
                            py[:],
                            hb[:, isl * P:(isl + 1) * P],
                            wds[:, isl * H + hn * 512: isl * H + hn * 512 + 512],
                            start=(isl == 0), stop=(isl == IT - 1))
                    nc.any.tensor_copy(
                        out=y_sb[:, s * H + hn * 512: s * H + hn * 512 + 512],
                        in_=py[:])

            # ---- combine: out[T,H] = sum_s ST_s.T @ Y_s  + hsh.T @ swd
            for tm in range(TN):
                for hn in range(HN):
                    pc = cpsum.tile([P, 512], F32, tag="pc")
                    for s in range(ns):
                        nc.tensor.matmul(
                            pc[:],
                            st_sb[:, s * T + tm * P: s * T + (tm + 1) * P],
                            y_sb[:, s * H + hn * 512: s * H + hn * 512 + 512],
                            start=(s == 0), stop=False)
                    nc.tensor.matmul(
                        pc[:],
                        hsh_sb[:, tm * P:(tm + 1) * P],
                        swd_sb[:, hn * 512:(hn + 1) * 512],
                        start=False, stop=True)
                    ob = opool.tile([P, 512], F32, tag="ob")
                    nc.any.tensor_copy(out=ob[:], in_=pc[:])
                    nc.sync.dma_start(
                        out=out_d[tm * P:(tm + 1) * P, hn * 512:(hn + 1) * 512],
                        in_=ob[:])

    nc.finalize()
    return nc


def _run(nc, in_maps):
    res = run_bass_kernel_spmd(nc, in_maps, core_ids=list(range(N_CORES)))
    out = np.zeros((T, H), np.float32)
    for r in res.results:
        out += r["out"]
    return out


def kernel(**inputs):
    in_maps, ns = _prepare(inputs)
    nc = _build(ns)
    return _run(nc, in_maps)


# revision 17
# speedup vs baseline: 1834.7972x; 1834.7972x over previous
"""DeepSeek-style MoE layer (64 routed experts, top-6 grouped routing, 2
shared experts) on 8 Trainium2 NeuronCores.

Strategy (expert-parallel, slot-tile formulation):
  * Host computes the gate/grouped-top-k routing exactly in fp32 numpy and
    packs each expert's assigned tokens into "slot tiles" of <=128 tokens.
    An expert with load > 128 contributes multiple tiles.  Tiles are
    bin-packed across the 8 cores (weights of a tile's expert are streamed
    to the core that owns the tile), NS tiles per core, uniform so one SPMD
    module serves all cores.
  * Device per core: for each slot tile, gate/up projections
    (h1T = wgT @ xT form, I on partitions), SiLU * up, down projection
    (Y[slot, H]), then a combine matmul  out[T,H] += ST[slot,T].T @ Y
    where ST carries the (renormalized * routed_scaling_factor) combine
    weights scattered per token.  The shared experts are tensor-parallel
    across cores (SI=1024 split into 8 slices of 128) and accumulate into
    the same PSUM tiles.  Everything is bf16 in / fp32 accumulate.
  * Host sums the 8 partial [T,H] outputs (the expert-parallel unshard).
"""
import numpy as np
import ml_dtypes

import concourse.bacc as bacc
import concourse.mybir as mybir
from concourse import tile
from concourse.bass_utils import run_bass_kernel_spmd

BF16 = ml_dtypes.bfloat16

T, H, E, I = 1024, 1024, 64, 512
NG, TKG, TOPK = 8, 3, 6
RSF = 2.5
P = 128
KT = H // P          # 8 k-tiles over hidden dim
IT = I // P          # 4 tiles over intermediate dim
HN = H // 512        # 2 output column chunks
TN = T // P          # 8 token tiles
N_CORES = 8
SI_SLICE = P         # shared-expert intermediate slice per core (2*512/8)


# ---------------------------------------------------------------- routing
def _route(x, gate_w, e_bias):
    """Exact fp32 replica of the reference noaux_tc grouped top-k."""
    logits = x.astype(np.float32) @ gate_w.astype(np.float32)
    scores = 1.0 / (1.0 + np.exp(-logits))
    sc = scores + e_bias
    g = sc.reshape(T, NG, E // NG)
    top2 = np.sort(g, axis=-1)[:, :, -2:].sum(-1)
    gidx = np.argsort(-top2, axis=-1)[:, :TKG]
    gmask = np.zeros((T, NG), bool)
    gmask[np.arange(T)[:, None], gidx] = True
    emask = np.repeat(gmask, E // NG, axis=1)
    masked = np.where(emask, sc, -np.inf)
    ids = np.argsort(-masked, axis=-1)[:, :TOPK]
    w = np.take_along_axis(scores, ids, axis=1)
    w = w / w.sum(-1, keepdims=True)
    return ids, w


def _pack(ids, w):
    """Assign exactly E//N_CORES experts to each core (one weight stream per
    expert, no duplicate weight DMA).  Experts with load > P ("big", <= 2P)
    are placed at positions 6/7 of the per-core order; overflow tokens go to
    slot tiles 8/9 which structurally reuse the SBUF weights of positions
    6/7 (wsel below), keeping the module uniform across cores.

    Returns (cores, ns, n_wsets, wsel): cores[c] = list of ns
    (weight_set_index_or_expert, tokens, weights) tiles; expert order per
    core in cores_experts[c]."""
    per_e = []
    for e in range(E):
        rows, cols = np.nonzero(ids == e)
        per_e.append((rows, w[rows, cols] * RSF))
    loads = np.array([len(r) for r, _ in per_e])
    epc = E // N_CORES                      # experts per core (8)
    if loads.max() <= 2 * P and (loads > P).sum() <= 2 * N_CORES:
        bigs = sorted([e for e in range(E) if loads[e] > P],
                      key=lambda e: -loads[e])
        smalls = sorted([e for e in range(E) if loads[e] <= P],
                        key=lambda e: -loads[e])
        core_exp = [[] for _ in range(N_CORES)]
        big_cnt = [0] * N_CORES
        slot_sum = [0] * N_CORES
        for e in bigs:
            c = min((i for i in range(N_CORES)
                     if big_cnt[i] < 2 and len(core_exp[i]) < epc),
                    key=lambda i: (big_cnt[i], slot_sum[i]))
            core_exp[c].append(e)
            big_cnt[c] += 1
            slot_sum[c] += loads[e]
        for e in smalls:
            c = min((i for i in range(N_CORES) if len(core_exp[i]) < epc),
                    key=lambda i: (slot_sum[i], len(core_exp[i])))
            core_exp[c].append(e)
            slot_sum[c] += loads[e]
        ns = epc + 2
        wsel = list(range(epc)) + [epc - 2, epc - 1]
        cores = []
        for c in range(N_CORES):
            exps = core_exp[c]
            order = [e for e in exps if loads[e] <= P] +                     [e for e in exps if loads[e] > P]
            order = order[:epc]
            # bigs occupy the tail; ensure they sit at positions 6/7
            tiles = []
            for s in range(epc):
                e = order[s]
                toks, wts = per_e[e]
                tiles.append((e, toks[:P], wts[:P]))
            for pos in (epc - 2, epc - 1):
                e = order[pos]
                toks, wts = per_e[e]
                tiles.append((e, toks[P:2 * P], wts[P:2 * P]))
            cores.append((order, tiles))
        return cores, ns, epc, wsel
    # fallback: generic tile bin-packing, one weight stream per tile
    tiles = []
    for e in range(E):
        rows, wts = per_e[e]
        for off in range(0, len(rows), P):
            tiles.append((e, rows[off:off + P], wts[off:off + P]))
    cores0 = [[] for _ in range(N_CORES)]
    for t in sorted(tiles, key=lambda z: -len(z[1])):
        c = min(range(N_CORES),
                key=lambda i: (len(cores0[i]), sum(len(z[1]) for z in cores0[i])))
        cores0[c].append(t)
    ns = max(len(c) for c in cores0)
    cores = []
    for c in range(N_CORES):
        ctiles = list(cores0[c])
        while len(ctiles) < ns:
            ctiles.append((0, np.zeros(0, np.int64), np.zeros(0, np.float32)))
        cores.append(([e for e, _, _ in ctiles], ctiles))
    return cores, ns, ns, list(range(ns))


def _prepare(inputs):
    """Host-side shard/dispatch: returns (in_maps, NS)."""
    x = np.asarray(inputs["hidden_states"], np.float32)
    w_gate = np.asarray(inputs["w_gate"], np.float32)
    w_up = np.asarray(inputs["w_up"], np.float32)
    w_down = np.asarray(inputs["w_down"], np.float32)
    sw_gate = np.asarray(inputs["sw_gate"], np.float32)
    sw_up = np.asarray(inputs["sw_up"], np.float32)
    sw_down = np.asarray(inputs["sw_down"], np.float32)

    ids, w = _route(x, np.asarray(inputs["gate_w"], np.float32),
                    np.asarray(inputs["e_bias"], np.float32))
    cores, ns, n_wsets, wsel = _pack(ids, w)

    # xt layout [P, KT*T]: xt[p, k*T + t] = x[t, k*P + p]
    xt = np.ascontiguousarray(
        x.T.reshape(KT, P, T).transpose(1, 0, 2).reshape(P, KT * T)).astype(BF16)
    # pre-reshaped expert weights in sbuf layout
    wg_sb_all = np.ascontiguousarray(
        w_gate.reshape(E, KT, P, I).transpose(0, 2, 1, 3).reshape(E, P, KT * I)).astype(BF16)
    wu_sb_all = np.ascontiguousarray(
        w_up.reshape(E, KT, P, I).transpose(0, 2, 1, 3).reshape(E, P, KT * I)).astype(BF16)
    wd_sb_all = np.ascontiguousarray(
        w_down.reshape(E, IT, P, H).transpose(0, 2, 1, 3).reshape(E, P, IT * H)).astype(BF16)

    in_maps = []
    for c in range(N_CORES):
        order, ctiles = cores[c]
        xg = np.zeros((KT, P, ns * P), np.float32)
        st = np.zeros((ns, P, T), BF16)
        wg_t = np.zeros((n_wsets, P, KT * I), BF16)
        wu_t = np.zeros((n_wsets, P, KT * I), BF16)
        wd_t = np.zeros((n_wsets, P, IT * H), BF16)
        for j in range(n_wsets):
            e = order[j] if j < len(order) else 0
            wg_t[j] = wg_sb_all[e]
            wu_t[j] = wu_sb_all[e]
            wd_t[j] = wd_sb_all[e]
        for s, (e, toks, wts) in enumerate(ctiles):
            n = len(toks)
            if n:
                xg[:, :, s * P:s * P + n] = x[toks].T.reshape(KT, P, n)
                st[s, np.arange(n), toks] = wts.astype(BF16)
        # xg layout [P, KT*ns*P]: xg2[p, k*ns*P + col] = x[tok(col), k*P + p]
        xg2 = np.ascontiguousarray(
            xg.transpose(1, 0, 2).reshape(P, KT * ns * P)).astype(BF16)
        # st layout [P, ns*T]
        st2 = np.ascontiguousarray(
            st.transpose(1, 0, 2).reshape(P, ns * T))
        sl = slice(c * SI_SLICE, (c + 1) * SI_SLICE)
        swg = np.ascontiguousarray(
            sw_gate[:, sl].reshape(KT, P, SI_SLICE).transpose(1, 0, 2)
            .reshape(P, KT * SI_SLICE)).astype(BF16)
        swu = np.ascontiguousarray(
            sw_up[:, sl].reshape(KT, P, SI_SLICE).transpose(1, 0, 2)
            .reshape(P, KT * SI_SLICE)).astype(BF16)
        in_maps.append({
            "xg": xg2,
            "st": st2,
            "wg": wg_t,
            "wu": wu_t,
            "wd": wd_t,
            "xt": xt,
            "swg": swg,
            "swu": swu,
            "swd": sw_down[sl, :].astype(BF16),
        })
    return in_maps, ns, n_wsets, wsel


# ----------------------------------------------------------------- device
def _build(ns, n_wsets, wsel, loop_n=1, skip_compute=False, skip_combine=False):
    """loop_n > 1 wraps the whole body in a device-side loop; used only for
    timing measurements (marginal cost per iteration = true exec time).
    skip_compute/skip_combine build reduced variants for perf bisection."""
    import contextlib
    BF = mybir.dt.bfloat16
    F32 = mybir.dt.float32
    SILU = mybir.ActivationFunctionType.Silu

    nc = bacc.Bacc(None, target_bir_lowering=False)
    xg_d = nc.declare_dram_parameter("xg", [P, KT * ns * P], BF, isOutput=False)
    st_d = nc.declare_dram_parameter("st", [P, ns * T], BF, isOutput=False)
    wg_d = nc.declare_dram_parameter("wg", [n_wsets, P, KT * I], BF, isOutput=False)
    wu_d = nc.declare_dram_parameter("wu", [n_wsets, P, KT * I], BF, isOutput=False)
    wd_d = nc.declare_dram_parameter("wd", [n_wsets, P, IT * H], BF, isOutput=False)
    xt_d = nc.declare_dram_parameter("xt", [P, KT * T], BF, isOutput=False)
    swg_d = nc.declare_dram_parameter("swg", [P, KT * SI_SLICE], BF, isOutput=False)
    swu_d = nc.declare_dram_parameter("swu", [P, KT * SI_SLICE], BF, isOutput=False)
    swd_d = nc.declare_dram_parameter("swd", [SI_SLICE, H], BF, isOutput=False)
    out_d = nc.declare_dram_parameter("out", [T, H], F32, isOutput=True)

    with tile.TileContext(nc) as tc:
        with tc.tile_pool(name="big", bufs=1) as big, \
             tc.tile_pool(name="wpool", bufs=3) as wpool, \
             tc.tile_pool(name="hpool", bufs=3) as hpool, \
             tc.tile_pool(name="opool", bufs=3) as opool, \
             tc.tile_pool(name="gup", bufs=4, space="PSUM") as gup, \
             tc.tile_pool(name="ypsum", bufs=2, space="PSUM") as ypsum, \
             tc.tile_pool(name="capool", bufs=2, space="PSUM") as ca, \
             tc.tile_pool(name="oapool", bufs=16) as oapool, \
             (tc.For_i(0, loop_n, 1) if loop_n > 1 else contextlib.nullcontext()):

            xt_sb = big.tile([P, KT * T], BF, tag="xt")
            nc.sync.dma_start(out=xt_sb[:], in_=xt_d[:])
            swg_sb = big.tile([P, KT * SI_SLICE], BF, tag="swg")
            nc.sync.dma_start(out=swg_sb[:], in_=swg_d[:])
            swu_sb = big.tile([P, KT * SI_SLICE], BF, tag="swu")
            nc.sync.dma_start(out=swu_sb[:], in_=swu_d[:])
            swd_sb = big.tile([P, H], BF, tag="swd")
            nc.sync.dma_start(out=swd_sb[:], in_=swd_d[:])
            xg_sb = big.tile([P, KT * ns * P], BF, tag="xg")
            nc.sync.dma_start(out=xg_sb[:], in_=xg_d[:])
            st_sb = big.tile([P, ns * T], BF, tag="st")
            nc.sync.dma_start(out=st_sb[:], in_=st_d[:])
            y_sb = big.tile([P, ns * H], BF, tag="y")
            hsh_sb = big.tile([P, T], BF, tag="hsh")

            # ---- shared experts (TP slice of SI on this core)
            for tn in range(T // 512 if not skip_compute else 0):
                pg = gup.tile([P, 512], F32, tag="pg")
                for k in range(KT):
                    nc.tensor.matmul(
                        pg[:],
                        swg_sb[:, k * SI_SLICE:(k + 1) * SI_SLICE],
                        xt_sb[:, k * T + tn * 512: k * T + tn * 512 + 512],
                        start=(k == 0), stop=(k == KT - 1))
                hg = hpool.tile([P, 512], F32, tag="hg")
                nc.scalar.activation(hg[:], pg[:], SILU)
                pu = gup.tile([P, 512], F32, tag="pg")
                for k in range(KT):
                    nc.tensor.matmul(
                        pu[:],
                        swu_sb[:, k * SI_SLICE:(k + 1) * SI_SLICE],
                        xt_sb[:, k * T + tn * 512: k * T + tn * 512 + 512],
                        start=(k == 0), stop=(k == KT - 1))
                nc.vector.tensor_mul(hsh_sb[:, tn * 512:(tn + 1) * 512], hg[:], pu[:])

            # ---- routed experts, one slot tile (<=128 tokens, one expert) at a
            # time; tiles with wsel[s] < s reuse already-resident weights
            wtiles = {}
            SPLIT = ns // 2
            oa_tiles = {}
            for s in range(ns):
                j = wsel[s]
                if j not in wtiles:
                    wgs = wpool.tile([P, KT * I], BF, tag="wg")
                    nc.sync.dma_start(out=wgs[:], in_=wg_d[j])
                    wus = wpool.tile([P, KT * I], BF, tag="wu")
                    nc.sync.dma_start(out=wus[:], in_=wu_d[j])
                    wds = wpool.tile([P, IT * H], BF, tag="wd")
                    nc.sync.dma_start(out=wds[:], in_=wd_d[j])
                    wtiles[j] = (wgs, wus, wds)
                else:
                    wgs, wus, wds = wtiles[j]
                if skip_compute:
                    continue

                pg = gup.tile([P, I], F32, tag="pg")
                for isl in range(IT):
                    for k in range(KT):
                        nc.tensor.matmul(
                            pg[:, isl * P:(isl + 1) * P],
                            wgs[:, k * I + isl * P: k * I + (isl + 1) * P],
                            xg_sb[:, (k * ns + s) * P: (k * ns + s + 1) * P],
                            start=(k == 0), stop=(k == KT - 1))
                hg = hpool.tile([P, I], F32, tag="hg")
                nc.scalar.activation(hg[:], pg[:], SILU)
                pu = gup.tile([P, I], F32, tag="pg")
                for isl in range(IT):
                    for k in range(KT):
                        nc.tensor.matmul(
                            pu[:, isl * P:(isl + 1) * P],
                            wus[:, k * I + isl * P: k * I + (isl + 1) * P],
                            xg_sb[:, (k * ns + s) * P: (k * ns + s + 1) * P],
                            start=(k == 0), stop=(k == KT - 1))
                hb = hpool.tile([P, I], BF, tag="hb")
                nc.vector.tensor_mul(hb[:], hg[:], pu[:])
                for hn in range(HN):
                    py = ypsum.tile([P, 512], F32, tag="py")
                    for isl in range(IT):
                        nc.tensor.matmul(
                            py[:],
                            hb[:, isl * P:(isl + 1) * P],
                            wds[:, isl * H + hn * 512: isl * H + hn * 512 + 512],
                            start=(isl == 0), stop=(isl == IT - 1))
                    nc.vector.tensor_copy(
                        out=y_sb[:, s * H + hn * 512: s * H + hn * 512 + 512],
                        in_=py[:])
                if s == SPLIT - 1 and not (skip_compute or skip_combine):
                    # group-A combine over tiles 0..SPLIT-1: overlaps the
                    # DMA-paced remainder of the slot loop (PE has slack)
                    for tm in range(TN):
                        for hn in range(HN):
                            pa = ca.tile([P, 512], F32, tag="ca")
                            for s2 in range(SPLIT):
                                nc.tensor.matmul(
                                    pa[:],
                                    st_sb[:, s2 * T + tm * P: s2 * T + (tm + 1) * P],
                                    y_sb[:, s2 * H + hn * 512: s2 * H + hn * 512 + 512],
                                    start=(s2 == 0), stop=(s2 == SPLIT - 1))
                            oa = oapool.tile([P, 512], BF, tag="oa")
                            nc.vector.tensor_copy(out=oa[:], in_=pa[:])
                            oa_tiles[(tm, hn)] = oa

            # ---- combine: out[T,H] = sum_s ST_s.T @ Y_s  + hsh.T @ swd
            if skip_compute or skip_combine:
                for tm in range(TN):
                    for hn in range(HN):
                        ob = opool.tile([P, 512], F32, tag="ob")
                        nc.any.memset(ob[:], 0.0)
                        nc.sync.dma_start(
                            out=out_d[tm * P:(tm + 1) * P, hn * 512:(hn + 1) * 512],
                            in_=ob[:])
            else:
              for tm in range(TN):
                for hn in range(HN):
                    pc = ca.tile([P, 512], F32, tag="ca")
                    for s in range(SPLIT, ns):
                        nc.tensor.matmul(
                            pc[:],
                            st_sb[:, s * T + tm * P: s * T + (tm + 1) * P],
                            y_sb[:, s * H + hn * 512: s * H + hn * 512 + 512],
                            start=(s == SPLIT), stop=False)
                    nc.tensor.matmul(
                        pc[:],
                        hsh_sb[:, tm * P:(tm + 1) * P],
                        swd_sb[:, hn * 512:(hn + 1) * 512],
                        start=False, stop=True)
                    ob = opool.tile([P, 512], F32, tag="ob")
                    nc.vector.tensor_tensor(
                        out=ob[:], in0=pc[:], in1=oa_tiles[(tm, hn)][:],
                        op=mybir.AluOpType.add)
                    nc.sync.dma_start(
                        out=out_d[tm * P:(tm + 1) * P, hn * 512:(hn + 1) * 512],
                        in_=ob[:])

    nc.finalize()
    return nc


def _run(nc, in_maps):
    res = run_bass_kernel_spmd(nc, in_maps, core_ids=list(range(N_CORES)))
    out = np.zeros((T, H), np.float32)
    for r in res.results:
        out += r["out"]
    return out


def kernel(**inputs):
    in_maps, ns, n_wsets, wsel = _prepare(inputs)
    nc = _build(ns, n_wsets, wsel)
    return _run(nc, in_maps)
